# revision 1
# baseline (speedup 1.0000x reference)
"""Ragged masked-softmax attention-energy kernel for 8 Trainium2 NeuronCores.

Reference computation (B2=512, L=1024, E=512):
    energy = questions @ W.T + b              [B2, L, E]
    scores = energy @ weight_vec              [B2, L]
    scores[l >= len] = -inf
    out = softmax(scores, axis=1)

Two algebraic facts make this memory-bound and ragged:
  * (q @ W.T + b) @ wv == q @ (W.T @ wv) + (b . wv); softmax is shift
    invariant so the (b . wv) scalar cancels. Only v = W.T @ wv (a [E]
    vector, computed on device) ever multiplies the big tensor.
  * tokens at positions >= len contribute exactly 0 to the output, so
    only ceil(len/128) 128-token tiles per row need to be loaded at all.

Host side (pure data layout, no math): rows are bin-packed across the 8
cores by tile count; each core receives a packed array of its [128, 512]
token tiles plus 0/-1e30 mask columns and 0/1 segment matrices that
encode the col->row mapping as *data*, keeping the SPMD program uniform
across cores. Device computes scores = q.v + mask via fused DVE
multiply-reduce, transposes score blocks with the PE, exponentiates on
ACT (softmax max-subtraction is skipped: scores are O(1) by
construction), reduces per-row sums with 0/1 segment matmuls, and
normalizes. Host scatters the packed [col, 128] probabilities back into
the zero-initialized [B2, L] output.
"""

import os
import sys

import numpy as np

if "/opt/trn_rl_repo" not in sys.path:
    sys.path.insert(0, "/opt/trn_rl_repo")

E = 512
P = 128
TPG = 8  # tiles per DMA group; one group = [128, TPG*512] = 2 MiB
NCORES = 8
NEG = -1.0e30

_NC_CACHE = {}
LAST_RESULT = None


def _schedule(lens, n_cores):
    """Assign rows to cores (LPT by tile count, <=128 rows/core)."""
    k = [(int(l) + P - 1) // P for l in lens]
    order = sorted(range(len(lens)), key=lambda r: -k[r])
    loads = [0] * n_cores
    rows_of = [[] for _ in range(n_cores)]
    for r in order:
        cands = [c for c in range(n_cores) if len(rows_of[c]) < P]
        c = min(cands, key=lambda i: (loads[i], len(rows_of[i])))
        rows_of[c].append(r)
        loads[c] += k[r]
    t_max = max(max(loads), 1)
    G = -(-t_max // TPG)
    S = G * TPG
    NB = -(-S // P)
    return rows_of, k, G, NB


def _pack(questions, lens, n_cores):
    B2, L, E_ = questions.shape
    assert E_ == E
    rows_of, k, G, NB = _schedule(lens, n_cores)
    S = G * TPG
    COLS = NB * P
    in_maps = []
    cols_meta = []
    for c in range(n_cores):
        cols = [(r, t) for r in rows_of[c] for t in range(k[r])]
        local = {r: i for i, r in enumerate(rows_of[c])}
        qp = np.zeros((G, P, TPG * E), np.float32)
        msk = np.full((P, COLS), NEG, np.float32)
        seg = np.zeros((P, COLS), np.float32)
        segT = np.zeros((P, COLS), np.float32)
        for s, (r, t) in enumerate(cols):
            g, j = divmod(s, TPG)
            ntok = min(P, int(lens[r]) - t * P)
            qp[g, :ntok, j * E:(j + 1) * E] = questions[r, t * P:t * P + ntok, :]
            msk[:ntok, s] = 0.0
            b_, m = divmod(s, P)
            li = local[r]
            seg[m, b_ * P + li] = 1.0
            segT[li, b_ * P + m] = 1.0
        in_maps.append({"qp": qp, "msk": msk, "seg": seg, "segT": segT})
        cols_meta.append(cols)
    return in_maps, cols_meta, G, NB


def _build_nc(G, NB, reps=1):
    from concourse import bacc, bass, tile

    mybir = bass.mybir
    dt = mybir.dt.float32
    Alu = mybir.AluOpType
    ActF = mybir.ActivationFunctionType
    S = G * TPG
    COLS = NB * P
    GPB = P // TPG  # groups per 128-col block

    nc = bacc.Bacc("TRN2", target_bir_lowering=False, debug=False,
                   num_devices=NCORES)
    qp = nc.declare_dram_parameter("qp", [G, P, TPG * E], dt, isOutput=False)
    msk = nc.declare_dram_parameter("msk", [P, COLS], dt, isOutput=False)
    seg = nc.declare_dram_parameter("seg", [P, COLS], dt, isOutput=False)
    segT = nc.declare_dram_parameter("segT", [P, COLS], dt, isOutput=False)
    iden = nc.declare_dram_parameter("iden", [P, P], dt, isOutput=False)
    wm = nc.declare_dram_parameter("wm", [E, E], dt, isOutput=False)
    wv = nc.declare_dram_parameter("wv", [4, P], dt, isOutput=False)
    # shape varies with reps so the jax persistent compile cache cannot
    # alias NEFFs of different-reps builds (the BIR is not in the HLO key)
    nc.declare_dram_parameter("stamp", [1, reps], dt, isOutput=False)
    probs = nc.declare_dram_parameter("probs", [COLS, P], dt, isOutput=True)

    with tile.TileContext(nc) as tc:
        with (
            tc.tile_pool(name="const", bufs=1) as const,
            tc.tile_pool(name="qpool", bufs=6) as qpool,
            tc.tile_pool(name="spool", bufs=2) as spool,
            tc.tile_pool(name="scratch", bufs=2) as scratch,
            tc.tile_pool(name="ppool", bufs=1) as ppool,
            tc.tile_pool(name="psum", bufs=1, space=bass.MemorySpace.PSUM) as psp,
            tc.tile_pool(name="psum2", bufs=2, space=bass.MemorySpace.PSUM) as psp2,
        ):
            iden_sb = const.tile([P, P], dt, tag="iden")
            nc.sync.dma_start(iden_sb[:], iden[:])
            msk_sb = const.tile([P, COLS], dt, tag="msk")
            nc.sync.dma_start(msk_sb[:], msk[:])
            seg_sb = const.tile([P, COLS], dt, tag="seg")
            nc.sync.dma_start(seg_sb[:], seg[:])
            segT_sb = const.tile([P, COLS], dt, tag="segT")
            nc.sync.dma_start(segT_sb[:], segT[:])
            w_sb = const.tile([P, 4 * E], dt, tag="wmat")
            for j in range(4):
                nc.sync.dma_start(w_sb[:, j * E:(j + 1) * E],
                                  wm[j * P:(j + 1) * P, :])
            wv4 = const.tile([4, P], dt, tag="wv4")
            nc.sync.dma_start(wv4[:], wv[:])

            # v = W.T @ wv on device, then broadcast to all 128 partitions.
            wvT_ps = psp.tile([P, 4], dt, tag="setup")
            nc.tensor.transpose(wvT_ps[:], wv4[:], iden_sb[0:4, 0:4])
            wvT_sb = const.tile([P, 4], dt, tag="wvT")
            nc.scalar.copy(wvT_sb[:], wvT_ps[:])
            v_ps = psp.tile([1, E], dt, tag="setup")
            with tc.tile_critical():
                for j in range(4):
                    nc.tensor.matmul(v_ps[:], wvT_sb[:, j:j + 1],
                                     w_sb[:, j * E:(j + 1) * E],
                                     start=(j == 0), stop=(j == 3))
            v_sb = const.tile([1, E], dt, tag="vrow")
            nc.scalar.copy(v_sb[:], v_ps[:])
            ones_sb = const.tile([1, P], dt, tag="ones")
            nc.vector.memset(ones_sb[:], 1.0)
            vrep_ps = psp.tile([P, E], dt, tag="setup")
            nc.tensor.matmul(vrep_ps[:], ones_sb[:], v_sb[:],
                             start=True, stop=True)
            vrep_sb = const.tile([P, E], dt, tag="vrep")
            nc.vector.tensor_copy(vrep_sb[:], vrep_ps[:])

            def one_pass():
                rowsum_parts = const.tile([P, NB], dt, tag="rsparts")
                pr_tiles = []
                for b in range(NB):
                    sc_b = spool.tile([P, P], dt, tag="scores")
                    nc.vector.memset(sc_b[:], NEG)
                    for g in range(b * GPB, min((b + 1) * GPB, G)):
                        qt = qpool.tile([P, TPG * E], dt, tag="q")
                        nc.sync.dma_start(qt[:], qp[g])
                        for j in range(TPG):
                            s = g * TPG + j
                            cl = s - b * P
                            # tensor_tensor_reduce is broken on this
                            # runtime: DVE multiply, then ACT copy with
                            # free-axis accumulate for the dot product
                            scr = scratch.tile([P, E], dt, tag="scr")
                            nc.vector.tensor_tensor(
                                out=scr[:],
                                in0=qt[:, j * E:(j + 1) * E],
                                in1=vrep_sb[:],
                                op=Alu.mult,
                            )
                            scr2 = scratch.tile([P, E], dt, tag="scr2")
                            nc.scalar.activation(
                                scr2[:], scr[:], ActF.Copy,
                                accum_out=sc_b[:, cl:cl + 1])
                    # apply length/padding mask
                    sc2_b = spool.tile([P, P], dt, tag="scores2")
                    nc.vector.tensor_tensor(
                        out=sc2_b[:], in0=sc_b[:],
                        in1=msk_sb[:, b * P:(b + 1) * P], op=Alu.add)
                    # block tail: transpose -> exp(+sum) -> segment row-sums
                    # (walrus: transpose matmul output must start at PSUM
                    # partition 0, so each 64-col chunk gets its own tile)
                    pr_b = ppool.tile([P, P], dt, tag=f"pr{b}")
                    se_b = ppool.tile([P, 1], dt, tag=f"se{b}")
                    for i in range(2):
                        pt_ps = psp2.tile([64, P], dt, tag=f"tps{i}")
                        nc.tensor.transpose(pt_ps[:],
                                            sc2_b[:, 64 * i:64 * (i + 1)],
                                            iden_sb[:])
                        nc.scalar.activation(
                            pr_b[64 * i:64 * (i + 1), :],
                            pt_ps[:],
                            ActF.Exp,
                            accum_out=se_b[64 * i:64 * (i + 1), 0:1])
                    rs_ps = psp2.tile([P, 1], dt, tag="v1")
                    nc.tensor.matmul(rs_ps[:], seg_sb[:, b * P:(b + 1) * P],
                                     se_b[:], start=True, stop=True)
                    nc.scalar.copy(rowsum_parts[:, b:b + 1], rs_ps[:])
                    pr_tiles.append(pr_b)

                rowsum = const.tile([P, 1], dt, tag="rowsum")
                nc.vector.reduce_sum(rowsum[:], rowsum_parts[:],
                                     axis=mybir.AxisListType.X)
                rowsum_eps = const.tile([P, 1], dt, tag="rowsum_eps")
                # keep unused-row reciprocals finite so 0-weight matmul
                # terms stay 0 instead of 0*inf
                nc.vector.tensor_scalar_add(rowsum_eps[:], rowsum[:], 1e-30)
                recip_sb = const.tile([P, 1], dt, tag="recip")
                nc.vector.reciprocal(recip_sb[:], rowsum_eps[:])

                for b in range(NB):
                    ex_ps = psp2.tile([P, 1], dt, tag="v1")
                    nc.tensor.matmul(ex_ps[:], segT_sb[:, b * P:(b + 1) * P],
                                     recip_sb[:], start=True, stop=True)
                    ex_sb = scratch.tile([P, 1], dt, tag="exs")
                    nc.scalar.copy(ex_sb[:], ex_ps[:])
                    fin = scratch.tile([P, P], dt, tag="fin")
                    nc.vector.tensor_scalar_mul(fin[:], pr_tiles[b][:],
                                                ex_sb[:])
                    nc.sync.dma_start(probs[b * P:(b + 1) * P, :], fin[:])

            for _rep in range(reps):
                one_pass()

    nc.compile()
    return nc


def kernel(**inputs):
    global LAST_RESULT
    from concourse.bass_utils import run_bass_kernel_spmd

    questions = np.ascontiguousarray(np.asarray(inputs["questions"], np.float32))
    lens = np.asarray(inputs["questions_lens"], np.int32)
    W = np.ascontiguousarray(np.asarray(inputs["W"], np.float32))
    wv = np.ascontiguousarray(np.asarray(inputs["weight_vec"], np.float32))
    B2, L, E_ = questions.shape

    in_maps, cols_meta, G, NB = _pack(questions, lens, NCORES)
    iden = np.eye(P, dtype=np.float32)
    wvr = np.ascontiguousarray(wv.reshape(4, P))
    for m in in_maps:
        m["iden"] = iden
        m["wm"] = W
        m["wv"] = wvr
        m["stamp"] = np.zeros((1, 1), np.float32)

    key = (G, NB)
    if key not in _NC_CACHE:
        _NC_CACHE[key] = _build_nc(G, NB)
    nc = _NC_CACHE[key]

    res = run_bass_kernel_spmd(nc, in_maps, list(range(NCORES)))
    LAST_RESULT = res

    out = np.zeros((B2, L), np.float32)
    for c in range(NCORES):
        pr = res.results[c]["probs"]
        for s, (r, t) in enumerate(cols_meta[c]):
            ntok = min(P, int(lens[r]) - t * P)
            out[r, t * P:t * P + ntok] = pr[s, :ntok]
    return out



# revision 4
# speedup vs baseline: 4.6811x; 4.6811x over previous
"""Ragged masked-softmax attention-energy kernel for 8 Trainium2 NeuronCores.

Reference computation (B2=512, L=1024, E=512):
    energy = questions @ W.T + b              [B2, L, E]
    scores = energy @ weight_vec              [B2, L]
    scores[l >= len] = -inf
    out = softmax(scores, axis=1)

Two algebraic facts make this memory-bound and ragged:
  * (q @ W.T + b) @ wv == q @ (W.T @ wv) + (b . wv); softmax is shift
    invariant so the (b . wv) scalar cancels. Only v = W.T @ wv (a [E]
    vector, computed on device) ever multiplies the big tensor.
  * tokens at positions >= len contribute exactly 0 to the output, so
    only ceil(len/128) 128-token tiles per row need to be loaded at all.

v2: the big tensor is packed in bf16 (tolerance is 2e-2; bf16 costs
~0.5% of a prob) and TRANSPOSED on the host into [e, tok] tiles so the
per-token dot products run on the TensorEngine: per 128-token column,
four accumulating matmuls with the q e-chunk as the (FWL-eligible bf16)
stationary operand and the matching vT chunk as a 1-column moving
operand. Scores land directly as [tok, col] fp32 PSUM blocks, so the
softmax tail (mask add, PE transpose, exp+accumulate, segment-matmul
row sums, normalize) is unchanged from v1. This halves HBM traffic and
removes the DVE multiply / ACT reduce streams that made v1
compute-bound (~690+610 ns per tile on the two slowest engines).

Host side (pure data layout, no math): rows are bin-packed across the 8
cores by tile count; each core receives a packed array of its [512, 128]
transposed token tiles plus 0/-1e30 mask columns and 0/1 segment
matrices that encode the col->row mapping as *data*, keeping the SPMD
program uniform across cores. Host scatters the packed [col, 128]
probabilities back into the zero-initialized [B2, L] output.
"""

import os
import sys

import numpy as np
import ml_dtypes

if "/opt/trn_rl_repo" not in sys.path:
    sys.path.insert(0, "/opt/trn_rl_repo")

E = 512
P = 128
CH = E // P  # e-chunks per tile
TPG = 16  # tiles per DMA group; one group = [128, TPG*512] bf16 = 2 MiB
NCORES = 8
NEG = -1.0e30
BF16 = ml_dtypes.bfloat16

_NC_CACHE = {}
LAST_RESULT = None


def _schedule(lens, n_cores):
    """Assign rows to cores (LPT by tile count, <=128 rows/core)."""
    k = [(int(l) + P - 1) // P for l in lens]
    order = sorted(range(len(lens)), key=lambda r: -k[r])
    loads = [0] * n_cores
    rows_of = [[] for _ in range(n_cores)]
    for r in order:
        cands = [c for c in range(n_cores) if len(rows_of[c]) < P]
        c = min(cands, key=lambda i: (loads[i], len(rows_of[i])))
        rows_of[c].append(r)
        loads[c] += k[r]
    t_max = max(max(loads), 1)
    G = -(-t_max // TPG)
    S = G * TPG
    NB = -(-S // P)
    return rows_of, k, G, NB


def _pack(questions, lens, n_cores):
    B2, L, E_ = questions.shape
    assert E_ == E
    rows_of, k, G, NB = _schedule(lens, n_cores)
    S = G * TPG
    COLS = NB * P
    in_maps = []
    cols_meta = []
    for c in range(n_cores):
        cols = [(r, t) for r in rows_of[c] for t in range(k[r])]
        local = {r: i for i, r in enumerate(rows_of[c])}
        # transposed tile layout: qp[g, e_in_chunk, slot*CH*P + j*P + tok]
        qp = np.zeros((G, P, TPG * E), BF16)
        qv = qp.reshape(G, P, TPG, CH, P)  # [g, e, slot, chunk, tok]
        msk = np.full((P, COLS), NEG, np.float32)
        seg = np.zeros((P, COLS), np.float32)
        segT = np.zeros((P, COLS), np.float32)
        for s, (r, t) in enumerate(cols):
            g, j = divmod(s, TPG)
            ntok = min(P, int(lens[r]) - t * P)
            qt = questions[r, t * P:t * P + ntok, :]  # [ntok, E] fp32
            # [ntok, CH, P] -> [e_in_chunk, chunk, tok]
            qv[g, :, j, :, :ntok] = qt.reshape(ntok, CH, P).transpose(2, 1, 0)
            msk[:ntok, s] = 0.0
            b_, m = divmod(s, P)
            li = local[r]
            seg[m, b_ * P + li] = 1.0
            segT[li, b_ * P + m] = 1.0
        in_maps.append({"qp": qp, "msk": msk, "seg": seg, "segT": segT})
        cols_meta.append(cols)
    return in_maps, cols_meta, G, NB


def _build_nc(G, NB, reps=1):
    from concourse import bacc, bass, tile

    mybir = bass.mybir
    dt = mybir.dt.float32
    dtb = mybir.dt.bfloat16
    Alu = mybir.AluOpType
    ActF = mybir.ActivationFunctionType
    S = G * TPG
    COLS = NB * P
    GPB = P // TPG  # groups per 128-col block

    nc = bacc.Bacc("TRN2", target_bir_lowering=False, debug=False,
                   num_devices=NCORES)
    qp = nc.declare_dram_parameter("qp", [G, P, TPG * E], dtb, isOutput=False)
    msk = nc.declare_dram_parameter("msk", [P, COLS], dt, isOutput=False)
    seg = nc.declare_dram_parameter("seg", [P, COLS], dt, isOutput=False)
    segT = nc.declare_dram_parameter("segT", [P, COLS], dt, isOutput=False)
    iden = nc.declare_dram_parameter("iden", [P, P], dt, isOutput=False)
    wm = nc.declare_dram_parameter("wm", [E, E], dt, isOutput=False)
    wv = nc.declare_dram_parameter("wv", [4, P], dt, isOutput=False)
    # shape varies with reps so the jax persistent compile cache cannot
    # alias NEFFs of different-reps builds (the BIR is not in the HLO key)
    nc.declare_dram_parameter("stamp", [1, reps], dt, isOutput=False)
    probs = nc.declare_dram_parameter("probs", [COLS, P], dt, isOutput=True)

    with tile.TileContext(nc) as tc:
        with (
            tc.tile_pool(name="const", bufs=1) as const,
            tc.tile_pool(name="qpool", bufs=5) as qpool,
            tc.tile_pool(name="spool", bufs=2) as spool,
            tc.tile_pool(name="scratch", bufs=2) as scratch,
            tc.tile_pool(name="ppool", bufs=1) as ppool,
            tc.tile_pool(name="psum", bufs=1, space=bass.MemorySpace.PSUM) as psp,
            tc.tile_pool(name="psum2", bufs=2, space=bass.MemorySpace.PSUM) as psp2,
            tc.tile_pool(name="psc", bufs=2, space=bass.MemorySpace.PSUM) as psc,
        ):
            iden_sb = const.tile([P, P], dt, tag="iden")
            nc.sync.dma_start(iden_sb[:], iden[:])
            msk_sb = const.tile([P, COLS], dt, tag="msk")
            nc.sync.dma_start(msk_sb[:], msk[:])
            seg_sb = const.tile([P, COLS], dt, tag="seg")
            nc.sync.dma_start(seg_sb[:], seg[:])
            segT_sb = const.tile([P, COLS], dt, tag="segT")
            nc.sync.dma_start(segT_sb[:], segT[:])
            w_sb = const.tile([P, 4 * E], dt, tag="wmat")
            for j in range(4):
                nc.sync.dma_start(w_sb[:, j * E:(j + 1) * E],
                                  wm[j * P:(j + 1) * P, :])
            wv4 = const.tile([4, P], dt, tag="wv4")
            nc.sync.dma_start(wv4[:], wv[:])

            # wvT[f_in_chunk, fchunk] = wv[f] on 128 partitions
            wvT_ps = psp.tile([P, 4], dt, tag="setup")
            nc.tensor.transpose(wvT_ps[:], wv4[:], iden_sb[0:4, 0:4])
            wvT_sb = const.tile([P, 4], dt, tag="wvT")
            nc.scalar.copy(wvT_sb[:], wvT_ps[:])
            # vT[e_in_chunk, echunk] = (W.T @ wv)[e], via 4x4 accumulating
            # matvecs: out[m, c] = sum_f W[f, c*128+m] * wv[f]
            vT_ps = psp.tile([P, CH], dt, tag="setup")
            with tc.tile_critical():
                for cch in range(CH):
                    for j in range(4):
                        nc.tensor.matmul(
                            vT_ps[:, cch:cch + 1],
                            w_sb[:, j * E + cch * P:j * E + (cch + 1) * P],
                            wvT_sb[:, j:j + 1],
                            start=(j == 0), stop=(j == 3))
            vT_bf = const.tile([P, CH], dtb, tag="vTbf")
            nc.vector.tensor_copy(vT_bf[:], vT_ps[:])

            def one_pass():
                rowsum_parts = const.tile([P, NB], dt, tag="rsparts")
                pr_tiles = []
                for b in range(NB):
                    sc_ps = psc.tile([P, P], dt, tag="scps")
                    # dummy cols (no matmul group) must read as NEG, and
                    # stale PSUM from earlier reps must not leak in
                    nc.vector.memset(sc_ps[:], NEG)
                    for g in range(b * GPB, min((b + 1) * GPB, G)):
                        qt = qpool.tile([P, TPG * E], dtb, tag="q")
                        nc.sync.dma_start(qt[:], qp[g])
                        for t in range(TPG):
                            s = g * TPG + t
                            cl = s - b * P
                            base = t * E
                            for j in range(CH):
                                nc.tensor.matmul(
                                    sc_ps[:, cl:cl + 1],
                                    qt[:, base + j * P:base + (j + 1) * P],
                                    vT_bf[:, j:j + 1],
                                    start=(j == 0), stop=(j == CH - 1))
                    # apply length/padding mask
                    sc2_b = spool.tile([P, P], dt, tag="scores2")
                    nc.vector.tensor_tensor(
                        out=sc2_b[:], in0=sc_ps[:],
                        in1=msk_sb[:, b * P:(b + 1) * P], op=Alu.add)
                    # block tail: transpose -> exp(+sum) -> segment row-sums
                    # (walrus: transpose matmul output must start at PSUM
                    # partition 0, so each 64-col chunk gets its own tile)
                    pr_b = ppool.tile([P, P], dt, tag=f"pr{b}")
                    se_b = ppool.tile([P, 1], dt, tag=f"se{b}")
                    for i in range(2):
                        pt_ps = psp2.tile([64, P], dt, tag="tps")
                        nc.tensor.transpose(pt_ps[:],
                                            sc2_b[:, 64 * i:64 * (i + 1)],
                                            iden_sb[:])
                        nc.scalar.activation(
                            pr_b[64 * i:64 * (i + 1), :],
                            pt_ps[:],
                            ActF.Exp,
                            accum_out=se_b[64 * i:64 * (i + 1), 0:1])
                    rs_ps = psp2.tile([P, 1], dt, tag="v1")
                    nc.tensor.matmul(rs_ps[:], seg_sb[:, b * P:(b + 1) * P],
                                     se_b[:], start=True, stop=True)
                    nc.scalar.copy(rowsum_parts[:, b:b + 1], rs_ps[:])
                    pr_tiles.append(pr_b)

                rowsum = const.tile([P, 1], dt, tag="rowsum")
                nc.vector.reduce_sum(rowsum[:], rowsum_parts[:],
                                     axis=mybir.AxisListType.X)
                rowsum_eps = const.tile([P, 1], dt, tag="rowsum_eps")
                # keep unused-row reciprocals finite so 0-weight matmul
                # terms stay 0 instead of 0*inf
                nc.vector.tensor_scalar_add(rowsum_eps[:], rowsum[:], 1e-30)
                recip_sb = const.tile([P, 1], dt, tag="recip")
                nc.vector.reciprocal(recip_sb[:], rowsum_eps[:])

                for b in range(NB):
                    ex_ps = psp2.tile([P, 1], dt, tag="v1")
                    nc.tensor.matmul(ex_ps[:], segT_sb[:, b * P:(b + 1) * P],
                                     recip_sb[:], start=True, stop=True)
                    ex_sb = scratch.tile([P, 1], dt, tag="exs")
                    nc.scalar.copy(ex_sb[:], ex_ps[:])
                    fin = scratch.tile([P, P], dt, tag="fin")
                    nc.vector.tensor_scalar_mul(fin[:], pr_tiles[b][:],
                                                ex_sb[:])
                    nc.sync.dma_start(probs[b * P:(b + 1) * P, :], fin[:])

            for _rep in range(reps):
                one_pass()

    nc.compile()
    return nc


def kernel(**inputs):
    global LAST_RESULT
    from concourse.bass_utils import run_bass_kernel_spmd

    questions = np.ascontiguousarray(np.asarray(inputs["questions"], np.float32))
    lens = np.asarray(inputs["questions_lens"], np.int32)
    W = np.ascontiguousarray(np.asarray(inputs["W"], np.float32))
    wv = np.ascontiguousarray(np.asarray(inputs["weight_vec"], np.float32))
    B2, L, E_ = questions.shape

    in_maps, cols_meta, G, NB = _pack(questions, lens, NCORES)
    iden = np.eye(P, dtype=np.float32)
    wvr = np.ascontiguousarray(wv.reshape(4, P))
    for m in in_maps:
        m["iden"] = iden
        m["wm"] = W
        m["wv"] = wvr
        m["stamp"] = np.zeros((1, 1), np.float32)

    key = (G, NB)
    if key not in _NC_CACHE:
        _NC_CACHE[key] = _build_nc(G, NB)
    nc = _NC_CACHE[key]

    res = run_bass_kernel_spmd(nc, in_maps, list(range(NCORES)))
    LAST_RESULT = res

    out = np.zeros((B2, L), np.float32)
    for c in range(NCORES):
        pr = res.results[c]["probs"]
        for s, (r, t) in enumerate(cols_meta[c]):
            ntok = min(P, int(lens[r]) - t * P)
            out[r, t * P:t * P + ntok] = pr[s, :ntok]
    return out
